# revision 58
# baseline (speedup 1.0000x reference)
"""Trainium2 Bass kernel for nn_CLF_CBF_QP_Net (CLF-CBF-QP controller net).

Strategy (pure data parallelism, 8 cores, 4096 samples each):
  Phase 1 (PE/ACT/DVE): tiny MLP forward passes + analytic gradient chains
    as shared-weight matmuls; assemble per-sample QP coefficients into a
    [128 partitions x 32 free]-per-scalar "QP layout" (sample s = 32*p + f).
  Phase 2 (DVE + ACT): 80 OSQP/ADMM iterations, algebraically specialized:
    the 6x6 KKT matrix M = A^T A + diag(P)+sigma*I has a 2x2+diag Schur
    structure, so the per-sample solve reduces to ~30 fused element-wise ops
    per iteration (no 6x6 inverse, no [B,8,6] tensors).
  Phase 3: epilogue (u, relaxation, Vdot, Hdot) + DMA out.

The sigma*x term in the ADMM rhs is dropped (sigma=1e-6; validated to move
the result by <3e-6 relative) which removes the x-state from the loop.
"""
import numpy as np
from contextlib import ExitStack

import concourse.bass as bass
import concourse.bacc as bacc
import concourse.tile as tile
from concourse import mybir

f32 = mybir.dt.float32
AL = mybir.AluOpType
AF = mybir.ActivationFunctionType

# ---- problem constants (hardcoded; kernel.py must be self-contained) ----
B = 32768
N_IN = 8
N_HID = 64
N_CTRL = 2
QP_ITERS = 80
NCORES = 8
BC = B // NCORES          # 4096 samples per core
P = 128                   # partitions
F = BC // P               # 32 free elems per comp-scalar
NW = BC // 512            # 8 sample windows of 512
NWH = 2048 // 512         # 4 windows per chunk-half

_f = np.float32
R12 = float(_f(1.0) / _f(1.2))
C_ = float(_f(1.0) + _f(R12) * _f(R12))          # 1 + 1/1.44
D_ = float(_f(2.0) + _f(1e-6))                   # bottom-block diagonal
K_ = float(_f(C_) * (_f(1.0) - _f(1.0) / _f(D_)))  # Schur scale
INVD = float(_f(1.0) / _f(D_))
CREL = float(_f(-0.5) * _f(INVD))                # relax = CREL*(xtq2+xtq3)
CDOT = float((_f(1.0) + _f(R12)) * _f(0.5))      # Vdot scale


def build_program(compile=True):
    """Emit the per-core SPMD Tile program. Returns the Bass object."""
    nc = bacc.Bacc("TRN2", target_bir_lowering=False)

    dt = f32
    di = lambda name, shape: nc.dram_tensor(name, shape, dt, kind="ExternalInput").ap()
    do = lambda name, shape: nc.dram_tensor(name, shape, dt, kind="ExternalOutput").ap()

    x = di("x", (BC, N_IN))
    VW1 = di("VW1", (N_HID, N_IN)); Vb1 = di("Vb1", (N_HID,))
    VW2 = di("VW2", (N_HID, N_HID)); Vb2 = di("Vb2", (N_HID,))
    HW1 = di("HW1", (N_HID, N_IN)); Hb1 = di("Hb1", (N_HID,))
    HW2 = di("HW2", (N_HID, N_HID)); Hb2 = di("Hb2", (N_HID,))
    HW3 = di("HW3", (N_HID, N_HID)); Hb3 = di("Hb3", (N_HID,))
    HW4 = di("HW4", (1, N_HID)); Hb4 = di("Hb4", (1,))
    G0 = di("G0", (N_IN, N_CTRL)); Km = di("K", (N_IN, N_CTRL))

    out_u = do("out_u", (BC, 2))
    out_relax = do("out_relax", (BC, 1))
    out_V = do("out_V", (BC,))
    out_Vdot = do("out_Vdot", (BC, 1, 1))
    out_H = do("out_H", (BC, 1))
    out_Hdot = do("out_Hdot", (BC, 1, 1))

    with ExitStack() as ctx:
        tc = ctx.enter_context(tile.TileContext(nc))
        sb = ctx.enter_context(tc.tile_pool(name="sb", bufs=1))
        ps = ctx.enter_context(tc.tile_pool(name="ps", bufs=5, space="PSUM"))

        T = lambda shape, tag: sb.tile(shape, dt, name=tag, tag=tag)

        # ---------------- weight / const loads ----------------
        # Queue policy: sync = L1-critical; scalar = L2-critical; gpsimd = rest.
        XT = T([N_IN, BC], "XT")            # x^T  [8, 4096]
        nc.sync.dma_start(out=XT, in_=x.rearrange("s c -> c s"))

        W1VT = T([N_IN, N_HID], "W1VT")     # VW1^T (lhsT for L1 fwd)
        nc.scalar.dma_start(out=W1VT, in_=VW1.rearrange("a b -> b a"))
        W1HT = T([N_IN, N_HID], "W1HT")
        nc.scalar.dma_start(out=W1HT, in_=HW1.rearrange("a b -> b a"))


        def bias_tile(tag, b_ap, q):
            # [128, 1] <- b duplicated on both chunk halves, ONE dma
            t = T([P, 1], tag)
            q.dma_start(out=t, in_=b_ap.rearrange("(a one) -> a one", one=1)
                        .unsqueeze(0).broadcast_to((2, N_HID, 1)))
            return t

        B1V = bias_tile("B1V", Vb1, nc.sync)
        B1H = bias_tile("B1H", Hb1, nc.sync)
        B2V = bias_tile("B2V", Vb2, nc.scalar)
        B2H = bias_tile("B2H", Hb2, nc.scalar)
        B3H = bias_tile("B3H", Hb3, nc.scalar)

        def blockdiag2(tag, w_ap, transpose, q):
            t = T([P, P], tag)
            nc.gpsimd.memset(t, 0.0)
            srcv = w_ap.rearrange("a b -> b a") if transpose else w_ap
            q.dma_start(out=t[0:64, 0:64], in_=srcv)
            q.dma_start(out=t[64:128, 64:128], in_=srcv)
            return t

        BD2V = blockdiag2("BD2V", VW2, True, nc.scalar)   # fwd L2 (VW2^T)
        BD2H = blockdiag2("BD2H", HW2, True, nc.scalar)
        BD3H = blockdiag2("BD3H", HW3, True, nc.gpsimd)
        BG2V = blockdiag2("BG2V", VW2, False, nc.gpsimd)  # grad chain (as-is)
        BG2H = blockdiag2("BG2H", HW2, False, nc.gpsimd)
        BG3H = blockdiag2("BG3H", HW3, False, nc.gpsimd)

        def blockdiag_narrow(tag, w_ap, ncol, nrow):
            t = T([P, 2 * ncol], tag)
            nc.gpsimd.memset(t, 0.0)
            nc.gpsimd.dma_start(out=t[0:64, 0:ncol], in_=w_ap)
            nc.gpsimd.dma_start(out=t[64:128, ncol:2 * ncol], in_=w_ap)
            return t

        BG1V = blockdiag_narrow("BG1V", VW1, N_IN, 64)   # [128,16] gradV out
        BG1H = blockdiag_narrow("BG1H", HW1, N_IN, 64)
        BD4H = blockdiag_narrow("BD4H", HW4.rearrange("a b -> b a"), 1, 64)

        HVONES = T([P, 2], "HVONES")   # 0.5-ones blockdiag for V reduce
        nc.gpsimd.memset(HVONES, 0.0)
        nc.vector.memset(HVONES[0:64, 0:1], 0.5)
        nc.vector.memset(HVONES[64:128, 1:2], 0.5)

        B4H = T([2, 1], "B4H")
        nc.gpsimd.dma_start(out=B4H, in_=Hb4.rearrange("(a one) -> a one", one=1)
                            .broadcast_to((2, 1)))
        G0S = T([N_IN, N_CTRL], "G0S")
        nc.gpsimd.dma_start(out=G0S, in_=G0)
        KS = T([N_IN, N_CTRL], "KS")
        nc.gpsimd.dma_start(out=KS, in_=Km)

        W4P = T([P, 1], "W4P")
        nc.gpsimd.dma_start(
            out=W4P,
            in_=HW4.rearrange("one c -> (one c)").rearrange("(a o) -> a o", o=1)
            .unsqueeze(0).broadcast_to((2, N_HID, 1)))

        # ---------------- forward + grad chains ----------------
        def psum(shape, tag="ps"):
            return ps.tile(shape, dt, name=tag, tag=tag)

        _dmaq = [nc.gpsimd, nc.scalar, nc.gpsimd, nc.sync]
        _dmaqi = [0]

        def dma(out, in_):
            eng = _dmaq[_dmaqi[0] % len(_dmaq)]
            _dmaqi[0] += 1
            eng.dma_start(out=out, in_=in_)

        CH2 = 2048
        WIN = 512

        def wsl(t, w):
            return t[:, WIN * w:WIN * (w + 1)]

        def layer1(w1t, bias, dst):
            # dst [128, 2048] = tanh(x @ W1^T + b), chunk-stacked
            for w in range(NWH):
                pt = psum([P, WIN])
                for h in range(2):
                    nc.tensor.matmul(
                        pt[64 * h:64 * (h + 1), :], w1t,
                        XT[:, 2048 * h + WIN * w: 2048 * h + WIN * (w + 1)],
                        start=True, stop=True)
                nc.scalar.activation(out=wsl(dst, w), in_=pt,
                                     func=AF.Tanh, bias=bias, scale=1.0)

        def layer2(bd, bias, src, dst):
            # dst = tanh(blockdiag(bd)^T @ src + b)
            for w in range(NWH):
                pt = psum([P, WIN])
                nc.tensor.matmul(pt, bd, wsl(src, w), start=True, stop=True)
                nc.scalar.activation(out=wsl(dst, w), in_=pt,
                                     func=AF.Tanh, bias=bias, scale=1.0)

        def square_to(src, dst, eng):
            for w in range(NWH):
                if eng is nc.scalar:
                    nc.scalar.square(out=wsl(dst, w), in_=wsl(src, w))
                else:
                    eng.tensor_tensor(out=wsl(dst, w), in0=wsl(src, w),
                                      in1=wsl(src, w), op=AL.mult)

        def one_minus(src, dst, eng):
            # dst = 1 - src  (TS fused: (src * -1) + 1) windowized
            for w in range(NWH):
                eng.tensor_scalar(out=wsl(dst, w), in0=wsl(src, w), scalar1=-1.0,
                                  scalar2=1.0, op0=AL.mult, op1=AL.add)

        def chain_mult(bd, src, om, dst):
            # dst = (blockdiag(bd)^T @ src) * om   (psum operand fused on DVE)
            for w in range(NWH):
                pw = psum([P, WIN])
                nc.tensor.matmul(pw, bd, wsl(src, w), start=True, stop=True)
                nc.vector.tensor_tensor(out=wsl(dst, w), in0=pw, in1=wsl(om, w),
                                        op=AL.mult)

        def grad_rows(bd1, src, dst, eng):
            for w in range(NWH):
                pg = psum([16, WIN])
                nc.tensor.matmul(pg, bd1, wsl(src, w), start=True, stop=True)
                if eng is nc.scalar:
                    nc.scalar.copy(out=wsl(dst, w), in_=pg)
                else:
                    eng.tensor_copy(out=wsl(dst, w), in_=pg)

        # Independent scratch sets so the two nets overlap freely.
        T1V = T([P, CH2], "T1V"); T2V = T([P, CH2], "T2V")
        SQV = T([P, CH2], "SQV"); OM2V = T([P, CH2], "OM2V")
        Z1V = T([P, CH2], "Z1V")   # pre-tanh L1 V (for LfV = -wt . z1)
        T1H = T([P, CH2], "T1H"); T2H = T([P, CH2], "T2H"); T3H = T([P, CH2], "T3H")
        SQH = T([P, CH2], "SQH"); OM2H = T([P, CH2], "OM2H"); OM3H = T([P, CH2], "OM3H")
        Z1H = T([P, CH2], "Z1H")
        PRV = T([P, CH2], "PRV")   # wt*z1 product scratch
        VROW = T([2, CH2], "VROW"); HROW = T([2, CH2], "HROW")
        SVROW = T([2, CH2], "SVROW"); SHROW = T([2, CH2], "SHROW")
        GVROW = T([4, CH2], "GVROW"); GHROW = T([4, CH2], "GHROW")
        GHVB = T([P, 4 * F], "GHVB")

        ONES2 = T([P, 2], "ONES2")   # 1.0-blockdiag for row reductions
        nc.gpsimd.memset(ONES2, 0.0)
        nc.vector.memset(ONES2[0:64, 0:1], 1.0)
        nc.vector.memset(ONES2[64:128, 1:2], 1.0)

        def layer1z(w1t, bias, dst, zdst):
            # dst = tanh(z), zdst = z = x @ W1^T (pre-bias), chunk-stacked
            for w in range(NWH):
                pt = psum([P, WIN])
                for h in range(2):
                    nc.tensor.matmul(
                        pt[64 * h:64 * (h + 1), :], w1t,
                        XT[:, 2048 * h + WIN * w: 2048 * h + WIN * (w + 1)],
                        start=True, stop=True)
                nc.scalar.activation(out=wsl(dst, w), in_=pt,
                                     func=AF.Tanh, bias=bias, scale=1.0)
                nc.vector.tensor_copy(out=wsl(zdst, w), in_=pt)

        def row_reduce(lhsT, src, row, eng):
            # row[2 or 4, 2048] = lhsT^T-blockdiag @ src, windowized
            for w in range(NWH):
                pr = psum([lhsT.shape[1], WIN])
                nc.tensor.matmul(pr, lhsT, wsl(src, w), start=True, stop=True)
                if eng is nc.scalar:
                    nc.scalar.copy(out=wsl(row, w), in_=pr)
                else:
                    eng.tensor_copy(out=wsl(row, w), in_=pr)

        # ---- layer 1 + 2 (both nets interleaved) ----
        layer1z(W1VT, B1V, T1V, Z1V)
        layer1z(W1HT, B1H, T1H, Z1H)
        layer2(BD2V, B2V, T1V, T2V)        # t2V
        layer2(BD2H, B2H, T1H, T2H)        # t2H
        layer2(BD3H, B3H, T2H, T3H)        # t3H

        # ---- squares / (1 - t^2) ----
        square_to(T2V, SQV, nc.scalar)     # t2V^2 (V-red + om2V)
        one_minus(SQV, OM2V, nc.vector)
        square_to(T3H, SQH, nc.scalar)
        one_minus(SQH, OM3H, nc.vector)    # om3H (-> a3 in-place below)

        # V row: 0.5 * sum_h t2^2 ;  H row: t3 @ HW4^T + b4
        row_reduce(HVONES, SQV, VROW, nc.scalar)
        row_reduce(BD4H, T3H, HROW, nc.scalar)
        nc.vector.tensor_scalar(out=HROW, in0=HROW, scalar1=B4H, scalar2=None,
                                op0=AL.add)

        # a2V = t2*om2 (in-place into T2V) ; a3H = om3*W4 (in-place)
        for w in range(NWH):
            nc.vector.tensor_tensor(out=wsl(T2V, w), in0=wsl(T2V, w),
                                    in1=wsl(OM2V, w), op=AL.mult)
        A2V = T2V
        for w in range(NWH):
            nc.vector.tensor_scalar(out=wsl(OM3H, w), in0=wsl(OM3H, w),
                                    scalar1=W4P, scalar2=None, op0=AL.mult)
        A3H = OM3H

        # remaining squares and oms
        square_to(T1V, SQV, nc.scalar)     # SQV free after V-row+om2V
        one_minus(SQV, SQV, nc.vector)     # om1V in place
        OM1V = SQV
        square_to(T2H, SQH, nc.scalar)     # SQH free after om3H
        one_minus(SQH, OM2H, nc.vector)
        square_to(T1H, SQH, nc.scalar)
        one_minus(SQH, SQH, nc.vector)     # om1H in place
        OM1H = SQH

        # ---- grad chains (no gradV/gradH materialization needed!) ----
        chain_mult(BG2V, A2V, OM1V, T1V)   # wtV = (a2 @ VW2) * om1V -> T1V
        WTV = T1V
        chain_mult(BG3H, A3H, OM2H, T3H)   # bH = (a3 @ HW3) * om2H -> T3H
        chain_mult(BG2H, T3H, OM1H, T2H)   # cH = (bH @ HW2) * om1H -> T2H
        CHt = T2H

        # LfV = -sum_n gradV*x = -wt . z1nb  (row reduction of wt*z1)
        for w in range(NWH):
            nc.vector.tensor_tensor(out=wsl(PRV, w), in0=wsl(WTV, w),
                                    in1=wsl(Z1V, w), op=AL.mult)
        row_reduce(ONES2, PRV, SVROW, nc.vector)   # SV-row = -LfV rows
        for w in range(NWH):
            nc.vector.tensor_tensor(out=wsl(PRV, w), in0=wsl(CHt, w),
                                    in1=wsl(Z1H, w), op=AL.mult)
        row_reduce(ONES2, PRV, SHROW, nc.vector)

        # VG0 = VW1 @ G0 (and HG0) -> blockdiag [128, 4]
        BVG0 = T([P, 4], "BVG0")
        nc.gpsimd.memset(BVG0, 0.0)
        pvg = psum([N_HID, 2])
        nc.tensor.matmul(pvg, W1VT, G0S, start=True, stop=True)
        nc.vector.tensor_copy(out=BVG0[0:64, 0:2], in_=pvg)
        nc.vector.tensor_copy(out=BVG0[64:128, 2:4], in_=pvg)
        BHG0 = T([P, 4], "BHG0")
        nc.gpsimd.memset(BHG0, 0.0)
        phg = psum([N_HID, 2])
        nc.tensor.matmul(phg, W1HT, G0S, start=True, stop=True)
        nc.vector.tensor_copy(out=BHG0[0:64, 0:2], in_=phg)
        nc.vector.tensor_copy(out=BHG0[64:128, 2:4], in_=phg)

        # gH/gV rows -> GHVB comp blocks [gH0|gH1|gV0|gV1]
        row_reduce(BVG0, WTV, GVROW, nc.scalar)
        row_reduce(BHG0, CHt, GHROW, nc.scalar)
        for (row, dst_lo) in ((GHROW, 0), (GVROW, 2)):
            for b in range(2):
                for cc in range(2):
                    dma(out=GHVB[64 * b:64 * (b + 1),
                                 (dst_lo + cc) * F:(dst_lo + cc + 1) * F],
                        in_=row[2 * b + cc:2 * b + cc + 1, :])

        # U2C = (x @ K)^T in QP layout [128, 2F] (unstacked windows)
        U2C = T([P, 2 * F], "U2C")
        U2ROW = T([2, BC], "U2ROW")
        for w in range(NW):
            pu = psum([2, WIN])
            nc.tensor.matmul(pu, KS, XT[:, WIN * w:WIN * (w + 1)],
                             start=True, stop=True)
            if w % 2 == 0:
                nc.scalar.copy(out=wsl(U2ROW, w), in_=pu)
            else:
                nc.vector.tensor_copy(out=wsl(U2ROW, w), in_=pu)
        for cc in range(2):
            dma(out=U2C[:, cc * F:(cc + 1) * F], in_=U2ROW[cc:cc + 1, :])

        # SV/SH to QP layout
        SV = T([P, F], "SV")     # sum_c gradV*x  (= -LfV)
        SH = T([P, F], "SH")
        for (row, dst) in ((SVROW, SV), (SHROW, SH)):
            for b in range(2):
                dma(out=dst[64 * b:64 * (b + 1), :], in_=row[b:b + 1, :])

        # ---- V/H to QP layout; bounds ----
        V_qp = T([P, F], "V_qp"); H_qp = T([P, F], "H_qp")
        for (row, dst) in ((VROW, V_qp), (HROW, H_qp)):
            for b in range(2):
                dma(out=dst[64 * b:64 * (b + 1), :], in_=row[b:b + 1, :])

        LOC = T([P, 2 * F], "LOC"); HIV = T([P, 2 * F], "HIV")
        # loC_m = -(LfH/m + H) = SH/m - H ;  hiV_m = SV/m - V
        nc.vector.tensor_tensor(out=LOC[:, 0:F], in0=SH, in1=H_qp, op=AL.subtract)
        nc.vector.scalar_tensor_tensor(out=LOC[:, F:2 * F], in0=SH, scalar=R12,
                                       in1=H_qp, op0=AL.mult, op1=AL.subtract)
        nc.vector.tensor_tensor(out=HIV[:, 0:F], in0=SV, in1=V_qp, op=AL.subtract)
        nc.vector.scalar_tensor_tensor(out=HIV[:, F:2 * F], in0=SV, scalar=R12,
                                       in1=V_qp, op0=AL.mult, op1=AL.subtract)

        # ---- Schur inverse coefficients ----
        gH0 = GHVB[:, 0:F]; gH1 = GHVB[:, F:2 * F]
        gV0 = GHVB[:, 2 * F:3 * F]; gV1 = GHVB[:, 3 * F:4 * F]
        gHpair = GHVB[:, 0:2 * F]; gVpair = GHVB[:, 2 * F:4 * F]
        gc4 = GHVB.rearrange("p (c f) -> p c f", c=4)
        GHV0 = gc4[:, 0::2, :]   # (gH0, gV0)
        GHV1 = gc4[:, 1::2, :]   # (gH1, gV1)

        def bc2(t_ap, j):
            return t_ap[:, j * F:(j + 1) * F].unsqueeze(1).broadcast_to((P, 2, F))

        PA = T([P, 2 * F], "PA"); PB = T([P, 2 * F], "PB")
        # (p00, p01) = gH0*(gH0,gH1) + gV0*(gV0,gV1)
        nc.vector.tensor_tensor(out=PA, in0=gHpair, in1=bc2(GHVB, 0), op=AL.mult)
        nc.vector.tensor_tensor(out=PB, in0=gVpair, in1=bc2(GHVB, 2), op=AL.mult)
        nc.vector.tensor_tensor(out=PA, in0=PA, in1=PB, op=AL.add)
        # p11 = gH1^2 + gV1^2
        P11 = T([P, F], "P11"); P11b = T([P, F], "P11b")
        nc.vector.tensor_tensor(out=P11, in0=gH1, in1=gH1, op=AL.mult)
        nc.vector.tensor_tensor(out=P11b, in0=gV1, in1=gV1, op=AL.mult)
        nc.vector.tensor_tensor(out=P11, in0=P11, in1=P11b, op=AL.add)
        # S = k*p + d (diag) / k*p01
        S3 = T([P, 3 * F], "S3")   # (S00, S01, S11)
        nc.vector.tensor_scalar(out=S3[:, 0:F], in0=PA[:, 0:F], scalar1=K_,
                                scalar2=D_, op0=AL.mult, op1=AL.add)
        nc.vector.tensor_scalar(out=S3[:, F:2 * F], in0=PA[:, F:2 * F], scalar1=K_,
                                scalar2=None, op0=AL.mult)
        nc.vector.tensor_scalar(out=S3[:, 2 * F:3 * F], in0=P11, scalar1=K_,
                                scalar2=D_, op0=AL.mult, op1=AL.add)
        DET = T([P, F], "DET"); DTB = T([P, F], "DTB")
        nc.vector.tensor_tensor(out=DET, in0=S3[:, 0:F], in1=S3[:, 2 * F:3 * F],
                                op=AL.mult)
        nc.vector.tensor_tensor(out=DTB, in0=S3[:, F:2 * F], in1=S3[:, F:2 * F],
                                op=AL.mult)
        nc.vector.tensor_tensor(out=DET, in0=DET, in1=DTB, op=AL.subtract)
        IDET = T([P, F], "IDET")
        nc.vector.reciprocal(out=IDET, in_=DET)
        SI = T([P, 3 * F], "SI")   # (Si00, Si01, Si11)
        nc.vector.tensor_tensor(out=SI[:, 0:F], in0=S3[:, 2 * F:3 * F], in1=IDET,
                                op=AL.mult)
        nc.vector.scalar_tensor_tensor(out=SI[:, F:2 * F], in0=S3[:, F:2 * F],
                                       scalar=-1.0, in1=IDET,
                                       op0=AL.mult, op1=AL.mult)
        nc.vector.tensor_tensor(out=SI[:, 2 * F:3 * F], in0=S3[:, 0:F], in1=IDET,
                                op=AL.mult)
        SiA = SI[:, 0:2 * F]       # (Si00, Si01)
        SiB = SI[:, F:3 * F]       # (Si01, Si11)

        # Fused per-sample coefficient tiles for the ADMM loop:
        #   GH4  = (gH0, gV0, gH1, gV1)   (pairs with bc4 broadcasts)
        #   GB4n = GH4 * (-1/d)           (Bv/d folded)
        #   R4   = (RH0, RH1, RV0, RV1) with (RHj) = Si-matvec of gH,
        #          so pH = RH0*e0 + RH1*e1 (xt01 drops out of the loop)
        GH4 = T([P, 4 * F], "GH4")
        nc.vector.tensor_copy(out=GH4.rearrange("p (c f) -> p c f", c=4)[:, 0::2, :],
                              in_=gHpair)
        nc.vector.tensor_copy(out=GH4.rearrange("p (c f) -> p c f", c=4)[:, 1::2, :],
                              in_=gVpair)
        GB4n = T([P, 4 * F], "GB4n")
        nc.vector.tensor_scalar(out=GB4n, in0=GH4, scalar1=-INVD, scalar2=None,
                                op0=AL.mult)
        R4 = T([P, 4 * F], "R4")
        RT1 = T([P, 2 * F], "RT1")
        nc.vector.tensor_tensor(out=RT1, in0=SiA, in1=bc2(GHVB, 0), op=AL.mult)
        nc.vector.tensor_tensor(out=R4[:, 0:2 * F], in0=SiB, in1=bc2(GHVB, 1),
                                op=AL.mult)
        nc.vector.tensor_tensor(out=R4[:, 0:2 * F], in0=R4[:, 0:2 * F], in1=RT1,
                                op=AL.add)
        nc.vector.tensor_tensor(out=RT1, in0=SiA, in1=bc2(GHVB, 2), op=AL.mult)
        nc.vector.tensor_tensor(out=R4[:, 2 * F:4 * F], in0=SiB, in1=bc2(GHVB, 3),
                                op=AL.mult)
        nc.vector.tensor_tensor(out=R4[:, 2 * F:4 * F], in0=R4[:, 2 * F:4 * F],
                                in1=RT1, op=AL.add)

        # ================= Phase 2: 80 ADMM iterations =================
        ZZ = T([P, 8 * F], "ZZ"); YY = T([P, 8 * F], "YY"); WW = T([P, 8 * F], "WW")
        SHV = T([P, 2 * F], "SHV")
        MA = T([P, 4 * F], "MA"); MB = T([P, 4 * F], "MB"); MC = T([P, 4 * F], "MC")
        RHS01 = T([P, 2 * F], "RHS01")
        RQ = T([P, 4 * F], "RQ")
        U12 = T([P, 2 * F], "U12")
        EPRE = T([P, 2 * F], "EPRE")
        E2 = T([P, 2 * F], "E2")
        XA = T([P, 2 * F], "XA"); XB = T([P, 2 * F], "XB")
        XT01 = T([P, 2 * F], "XT01")
        P4 = T([P, 4 * F], "P4")
        XTQ = T([P, 4 * F], "XTQ")
        ZTQ = T([P, 4 * F], "ZTQ"); TQ = T([P, 4 * F], "TQ")
        T47 = T([P, 4 * F], "T47")

        nc.vector.memset(ZZ, 0.0)
        nc.vector.memset(YY, 0.0)
        nc.vector.memset(WW, 0.0)

        # comp views (all <= partition + 2 free dims for the BIR verifier)
        ww4 = WW.rearrange("p (g c f) -> p g c f", g=2, c=4)
        wEV = ww4[:, 0, 0::2, :]            # w comps (0, 2)
        wOD = ww4[:, 0, 1::2, :]            # w comps (1, 3)
        rq4 = RQ.rearrange("p (c f) -> p c f", c=4)
        # RQ layout: [rhs4, rhs2n, rhs5, rhs3n] (matches P4 = [pH, pV, pHr, pVr])
        rqEV = rq4[:, 0::2, :]              # (rhs4, rhs5)
        rqOD = rq4[:, 1::2, :]              # (rhs2n, rhs3n)
        tq4 = TQ.rearrange("p (c f) -> p c f", c=4)
        tqEV = tq4[:, 0::2, :]              # (t0, t2): CBF rows
        tqOD = tq4[:, 1::2, :]              # (t1, t3): CLF rows
        xtq4 = XTQ.rearrange("p (c f) -> p c f", c=4)
        xtqEV = xtq4[:, 0::2, :]            # d*(xt4, xt5)
        xtqOD = xtq4[:, 1::2, :]            # d*(-xt2, -xt3)
        zz4 = ZZ.rearrange("p (g c f) -> p g c f", g=2, c=4)
        zzEV = zz4[:, 0, 0::2, :]
        zzOD = zz4[:, 0, 1::2, :]
        yy4 = YY.rearrange("p (g c f) -> p g c f", g=2, c=4)
        yyEV = yy4[:, 0, 0::2, :]
        yyOD = yy4[:, 0, 1::2, :]
        maEV = MA.rearrange("p (c f) -> p c f", c=4)[:, 0::2, :]
        maOD = MA.rearrange("p (c f) -> p c f", c=4)[:, 1::2, :]
        mbEV = MB.rearrange("p (c f) -> p c f", c=4)[:, 0::2, :]
        mbOD = MB.rearrange("p (c f) -> p c f", c=4)[:, 1::2, :]
        mcEV = MC.rearrange("p (c f) -> p c f", c=4)[:, 0::2, :]
        mcOD = MC.rearrange("p (c f) -> p c f", c=4)[:, 1::2, :]

        def bc4(t_ap):
            # [P, 2F] tile -> (a, b, a, b) [P, 2, 2F] broadcast view
            return t_ap.unsqueeze(1).broadcast_to((P, 2, 2 * F))

        STT = nc.vector.scalar_tensor_tensor
        TT = nc.vector.tensor_tensor

        for it in range(QP_ITERS):
            # sHV = w[0:2] + r12 * w[2:4]
            STT(out=SHV, in0=WW[:, 2 * F:4 * F], scalar=R12, in1=WW[:, 0:2 * F],
                op0=AL.mult, op1=AL.add)
            # rhs01 = gH*sH + gV*sV + 2*u_nom   (q01 = -2 u_nom)
            TT(out=MA, in0=GH4, in1=bc4(SHV), op=AL.mult)
            TT(out=RHS01, in0=maEV, in1=maOD, op=AL.add)
            STT(out=RHS01, in0=U2C, scalar=-2.0, in1=RHS01, op0=AL.mult, op1=AL.add)
            # rq even comps (rhs4, rhs5) = (w02 - 50) + w67
            STT(out=rqEV, in0=wEV, scalar=-50.0, in1=WW[:, 6 * F:8 * F],
                op0=AL.add, op1=AL.add)
            # rq odd comps (rhs2n, rhs3n) = (w13 + 100) - w45
            STT(out=rqOD, in0=wOD, scalar=100.0, in1=WW[:, 4 * F:6 * F],
                op0=AL.add, op1=AL.subtract)
            # U12 = (u2, u1n): u2 = r12*rhs5 + rhs4 ; u1n = r12*rhs3n + rhs2n
            STT(out=U12, in0=RQ[:, 2 * F:4 * F], scalar=R12, in1=RQ[:, 0:2 * F],
                op0=AL.mult, op1=AL.add)
            # e = rhs01 - Bv/d:  MB = GB4n*(u2,u1n,u2,u1n); pair-sum = -Bv/d
            TT(out=MB, in0=GB4n, in1=bc4(U12), op=AL.mult)
            TT(out=EPRE, in0=mbEV, in1=mbOD, op=AL.add)
            TT(out=E2, in0=RHS01, in1=EPRE, op=AL.add)
            # P4 = (pH, pV, pHr, pVr) directly from e via fused R4 coeffs
            TT(out=MC, in0=R4, in1=bc4(E2), op=AL.mult)
            TT(out=P4[:, 0:2 * F], in0=mcEV, in1=mcOD, op=AL.add)
            nc.vector.tensor_scalar(out=P4[:, 2 * F:4 * F], in0=P4[:, 0:2 * F],
                                    scalar1=R12, scalar2=None, op0=AL.mult)
            # xtq' = rq - P4 = d*(xt4, -xt2, xt5, -xt3)
            TT(out=XTQ, in0=RQ, in1=P4, op=AL.subtract)
            # ztq = xtq'/d + P4 = (zt0, zt1, zt2, zt3)  (natural order!)
            STT(out=ZTQ, in0=XTQ, scalar=INVD, in1=P4, op0=AL.mult, op1=AL.add)
            # tq = ztq + yy[0:4]
            TT(out=TQ, in0=ZTQ, in1=YY[:, 0:4 * F], op=AL.add)
            # t45 = yy45 - (-xtqOD)/d ; t67 = yy67 + xtqEV/d
            STT(out=T47[:, 0:2 * F], in0=xtqOD, scalar=-INVD,
                in1=YY[:, 4 * F:6 * F], op0=AL.mult, op1=AL.add)
            STT(out=T47[:, 2 * F:4 * F], in0=xtqEV, scalar=INVD,
                in1=YY[:, 6 * F:8 * F], op0=AL.mult, op1=AL.add)
            # clips: CBF rows (even comps) lower-bounded, CLF rows upper-bounded
            TT(out=zzEV, in0=tqEV, in1=LOC, op=AL.max)
            TT(out=zzOD, in0=tqOD, in1=HIV, op=AL.min)
            nc.vector.tensor_scalar_max(out=ZZ[:, 4 * F:8 * F], in0=T47,
                                        scalar1=0.0)
            # y updates
            TT(out=yyEV, in0=tqEV, in1=zzEV, op=AL.subtract)
            TT(out=yyOD, in0=tqOD, in1=zzOD, op=AL.subtract)
            TT(out=YY[:, 4 * F:8 * F], in0=T47, in1=ZZ[:, 4 * F:8 * F],
               op=AL.subtract)
            if it < QP_ITERS - 1:
                TT(out=WW, in0=ZZ, in1=YY, op=AL.subtract)

        # xt01 for outputs, from the final iteration's e
        TT(out=XA, in0=SiA, in1=bc2(E2, 0), op=AL.mult)
        TT(out=XB, in0=SiB, in1=bc2(E2, 1), op=AL.mult)
        TT(out=XT01, in0=XA, in1=XB, op=AL.add)

        # ================= Phase 3: epilogue =================
        REL = T([P, F], "REL")
        # relax = (xt2+xt3)/2 = -(XTQ[1]+XTQ[3])/(2d)
        TT(out=REL, in0=XTQ[:, F:2 * F], in1=XTQ[:, 3 * F:4 * F], op=AL.add)
        nc.vector.tensor_scalar(out=REL, in0=REL, scalar1=CREL, scalar2=None,
                                op0=AL.mult)
        GP = T([P, 2 * F], "GP")
        GVU = T([P, F], "GVU"); GHU = T([P, F], "GHU")
        TT(out=GP, in0=gVpair, in1=XT01, op=AL.mult)
        TT(out=GVU, in0=GP[:, 0:F], in1=GP[:, F:2 * F], op=AL.add)
        TT(out=GP, in0=gHpair, in1=XT01, op=AL.mult)
        TT(out=GHU, in0=GP[:, 0:F], in1=GP[:, F:2 * F], op=AL.add)
        VD = T([P, F], "VD"); HD = T([P, F], "HD")
        TT(out=VD, in0=GVU, in1=SV, op=AL.subtract)
        nc.vector.tensor_scalar(out=VD, in0=VD, scalar1=CDOT, scalar2=None,
                                op0=AL.mult)
        TT(out=HD, in0=GHU, in1=SH, op=AL.subtract)
        nc.vector.tensor_scalar(out=HD, in0=HD, scalar1=CDOT, scalar2=None,
                                op0=AL.mult)

        # ---- output DMAs ----
        u_view = out_u.rearrange("(p f) c -> p c f", p=P)
        for cc in range(2):
            nc.scalar.dma_start(out=u_view[:, cc:cc + 1, :],
                                in_=XT01[:, cc * F:(cc + 1) * F])
        nc.gpsimd.dma_start(out=out_relax.rearrange("(p f) c -> p (c f)", p=P),
                            in_=REL)
        nc.sync.dma_start(out=out_V.rearrange("(c s) -> c s", c=2), in_=VROW)
        nc.scalar.dma_start(out=out_H.rearrange("(c s) one -> c (one s)", c=2),
                            in_=HROW)
        nc.gpsimd.dma_start(out=out_Vdot.rearrange("(p f) a b -> p (a b f)", p=P),
                            in_=VD)
        nc.gpsimd.dma_start(out=out_Hdot.rearrange("(p f) a b -> p (a b f)", p=P),
                            in_=HD)

    if compile:
        nc.compile()
    return nc


_cached = {}


def _get_program():
    if "nc" not in _cached:
        _cached["nc"] = build_program()
    return _cached["nc"]


def kernel(**inputs):
    from concourse import bass_utils

    nc = _get_program()
    weights = {k: np.ascontiguousarray(np.asarray(v, dtype=np.float32))
               for k, v in inputs.items() if k != "x"}
    x = np.ascontiguousarray(np.asarray(inputs["x"], dtype=np.float32))
    in_maps = []
    for i in range(NCORES):
        m = dict(weights)
        m["x"] = x[i * BC:(i + 1) * BC]
        in_maps.append(m)

    res = bass_utils.run_bass_kernel_spmd(nc, in_maps, core_ids=list(range(NCORES)))
    r = res.results
    cat = lambda name: np.concatenate([r[i][name] for i in range(NCORES)], axis=0)
    return (cat("out_u"), cat("out_relax"), cat("out_V"),
            cat("out_Vdot"), cat("out_H"), cat("out_Hdot"))
